# revision 11
# baseline (speedup 1.0000x reference)
"""Quantized-AlexNet forward on 8 trn2 NeuronCores.

Strategy:
  - data-parallel conv stack: 8 images per core
  - conv1 (not quantized): host im2col, fp32 matmul (exact)
  - conv2..5, fc1, fc2 (1-bit quantized): fp32r matmuls with exact +-1 sign
    weights; the DoReFa scale E (and BN affine) folded into the fp32
    activation epilogue -> only activation-rounding error (~1e-4)
  - conv3..5 run layer-wise over all 4 image-pairs with streamed weights
    (keeps SBUF under budget); rhs batches 2 images -> N=364 (fp32r needs
    an even innermost free dim, hence the 14-wide x overcompute)
  - maxpools: 2-pass DVE pool (x-window then y-window)
  - FC: tensor-parallel over the output dim with DRAM AllGathers between
    layers; activations kept output-transposed [feat, img] so biases are
    per-partition; h is PE-transposed once
"""

import os
import numpy as np

NCORES = 8
B = 64
BPC = B // NCORES  # images per core

_PROG_CACHE = {}
LAST_EXEC_NS = None
LAST_RESULTS = None


def _build_program():
    import concourse.bass as bass
    import concourse.mybir as mybir
    import concourse.tile as tile
    from concourse import bacc
    from concourse.masks import make_identity

    F32 = mybir.dt.float32
    F32R = mybir.dt.float32r
    AF = mybir.ActivationFunctionType

    def mkap(tile_ap, offset_elems, dims):
        part = tile_ap.ap[0]
        return bass.AP(
            tensor=tile_ap.tensor,
            offset=tile_ap.offset + offset_elems,
            ap=[list(part)] + [list(d) for d in dims],
        )

    def rawap(tile_ap, offset_elems, dims):
        return bass.AP(
            tensor=tile_ap.tensor,
            offset=tile_ap.offset + offset_elems,
            ap=[list(d) for d in dims],
        )

    nc = bacc.Bacc("TRN2", target_bir_lowering=False, debug=False,
                   num_devices=NCORES)

    def max3(out_ap, mk_in, step):
        nc.vector.tensor_max(out_ap, mk_in(0), mk_in(step))
        nc.vector.tensor_max(out_ap, out_ap, mk_in(2 * step))

    # ---- DRAM I/O ----
    xcol_d = nc.dram_tensor("xcol", [BPC, 3, 121, 3025], F32, kind="ExternalInput").ap()
    w1c_d = nc.dram_tensor("w1c", [3, 121, 96], F32, kind="ExternalInput").ap()
    sc1_d = nc.dram_tensor("sc1", [96, 2], F32, kind="ExternalInput").ap()
    w2c_d = nc.dram_tensor("w2c", [25, 96, 256], F32, kind="ExternalInput").ap()
    sc2_d = nc.dram_tensor("sc2", [256, 2], F32, kind="ExternalInput").ap()
    w3c_d = nc.dram_tensor("w3c", [9, 2, 128, 384], F32, kind="ExternalInput").ap()
    sc3_d = nc.dram_tensor("sc3", [384, 2], F32, kind="ExternalInput").ap()
    w4c_d = nc.dram_tensor("w4c", [9, 3, 128, 384], F32, kind="ExternalInput").ap()
    sc4_d = nc.dram_tensor("sc4", [384, 2], F32, kind="ExternalInput").ap()
    w5c_d = nc.dram_tensor("w5c", [9, 3, 128, 256], F32, kind="ExternalInput").ap()
    sc5_d = nc.dram_tensor("sc5", [256, 2], F32, kind="ExternalInput").ap()
    fw1_d = nc.dram_tensor("fw1t", [9216, 512], F32, kind="ExternalInput").ap()
    sf1_d = nc.dram_tensor("sf1", [512, 2], F32, kind="ExternalInput").ap()
    fw2_d = nc.dram_tensor("fw2t", [4096, 512], F32, kind="ExternalInput").ap()
    sf2_d = nc.dram_tensor("sf2", [512, 2], F32, kind="ExternalInput").ap()
    fw3_d = nc.dram_tensor("fw3t", [4096, 126], F32, kind="ExternalInput").ap()
    sf3_d = nc.dram_tensor("sf3", [126, 2], F32, kind="ExternalInput").ap()
    out_d = nc.dram_tensor("out", [126, 64], F32, kind="ExternalOutput").ap()

    # conv1 flat column chunks over the 3025 output positions
    C1CHUNKS = [(0, 512), (512, 1024), (1024, 1536), (1536, 2048),
                (2048, 2560), (2560, 3025)]

    with tile.TileContext(nc) as tc:
        with tc.tile_pool(name="wp", bufs=1) as wp, \
             tc.tile_pool(name="dr", bufs=1, space="DRAM") as dr:

            # ---- persistent small tiles ----
            w1s = []
            for c in range(3):
                t = wp.tile([121, 96], F32, name=f"w1s{c}")
                nc.sync.dma_start(out=t, in_=w1c_d[c])
                w1s.append(t)

            def load_sc(src, M, name):
                ts = []
                for m in range((M + 127) // 128):
                    mm = min(128, M - m * 128)
                    t = wp.tile([128, 2], F32, name=f"{name}_{m}")
                    nc.sync.dma_start(out=t[:mm], in_=src[m * 128:m * 128 + mm])
                    ts.append(t)
                return ts

            sc1 = load_sc(sc1_d, 96, "sc1")
            sc2 = load_sc(sc2_d, 256, "sc2")
            sc3 = load_sc(sc3_d, 384, "sc3")
            sc4 = load_sc(sc4_d, 384, "sc4")
            sc5 = load_sc(sc5_d, 256, "sc5")
            sf1 = load_sc(sf1_d, 512, "sf1")
            sf2 = load_sc(sf2_d, 512, "sf2")
            sf3 = load_sc(sf3_d, 126, "sf3")

            w2s = []
            for sh in range(25):
                t = wp.tile([96, 256], F32R, name=f"w2s{sh}")
                nc.sync.dma_start(out=t, in_=w2c_d[sh].bitcast(F32R))
                w2s.append(t)

            idn = wp.tile([64, 64], F32)
            make_identity(nc, idn)
            zk = wp.tile([128, 1], F32)
            nc.vector.memset(zk, 0.0)

            def zfill(t, n):
                bc = bass.AP(tensor=zk.tensor, offset=zk.offset,
                             ap=[[zk.ap[0][0], t.ap[0][1]], [0, n]])
                nc.scalar.activation(t, bc, AF.Copy, scale=0.0)

            h_loc = dr.tile([BPC, 9216], F32)
            h_all = dr.tile([B, 9216], F32, addr_space="Shared")
            y1loc = dr.tile([512, 64], F32)
            y1all = dr.tile([4096, 64], F32, addr_space="Shared")
            y2loc = dr.tile([512, 64], F32)
            y2all = dr.tile([4096, 64], F32, addr_space="Shared")

            # ================= conv stack =================
            with tc.tile_pool(name="xc", bufs=9) as xc, \
                 tc.tile_pool(name="wq", bufs=12) as wq, \
                 tc.tile_pool(name="act", bufs=1) as act, \
                 tc.tile_pool(name="ps", bufs=8, space="PSUM") as ps:

                c3in = {}
                # ---- per-image: conv1, pool1, conv2, pool2 ----
                for i in range(BPC):
                    g, islot = divmod(i, 2)

                    c1out = act.tile([96, 3025], F32, tag="c1out", bufs=2,
                                     name=f"c1out_{i}")
                    for (c0, c1) in C1CHUNKS:
                        n = c1 - c0
                        xts = []
                        for c in range(3):
                            xt = xc.tile([121, 512], F32, tag="xc")
                            nc.sync.dma_start(out=xt[:, :n],
                                              in_=xcol_d[i, c, :, c0:c1])
                            xts.append(xt)
                        pt = ps.tile([128, 512], F32, tag="ps")
                        for c in range(3):
                            nc.tensor.matmul(pt[:96, :n], w1s[c], xts[c][:, :n],
                                             start=(c == 0), stop=(c == 2))
                        nc.scalar.activation(c1out[:, c0:c1], pt[:96, :n],
                                             AF.Relu, bias=sc1[0][:96, 1:2],
                                             scale=1.0)

                    p1 = act.tile([96, 55 * 27], F32, tag="p1", bufs=2,
                                  name=f"p1_{i}")
                    max3(mkap(p1, 0, [[27, 55], [1, 27]]),
                         lambda o: mkap(c1out, o, [[55, 55], [2, 27]]), 1)
                    c2in = act.tile([96, 31 * 32], F32R, tag="c2in", bufs=2)
                    zfill(c2in, 31 * 32)
                    c2wv = mkap(c2in, 2 * 32 + 2, [[1, 27], [32, 27]])
                    c2rv = mkap(c2in.bitcast(F32), 2 * 32 + 2, [[1, 27], [32, 27]])
                    nc.vector.tensor_max(c2wv, mkap(p1, 0, [[1, 27], [54, 27]]),
                                         mkap(p1, 27, [[1, 27], [54, 27]]))
                    nc.vector.tensor_max(c2wv, c2rv, mkap(p1, 54, [[1, 27], [54, 27]]))

                    # conv2 (fp32r, 25 shifts)
                    if islot == 0:
                        c3in[g] = [act.tile([128, 2 * 240], F32R, tag=f"c3in{kb}_{g}",
                                            name=f"c3in{kb}_{g}")
                                   for kb in range(2)]
                        for t in c3in[g]:
                            zfill(t, 2 * 240)
                    for m in range(2):
                        co = act.tile([128, 27 * 28], F32, tag="c2out", bufs=3)
                        for (y0, ycnt) in [(0, 14), (14, 13)]:
                            pt = ps.tile([128, 512], F32, tag="ps")
                            n = ycnt * 28
                            for sh in range(25):
                                r, s = divmod(sh, 5)
                                rhs = mkap(c2in, (y0 + r) * 32 + s,
                                           [[32, ycnt], [1, 28]])
                                nc.tensor.matmul(pt[:, :n],
                                                 w2s[sh][:, m * 128:(m + 1) * 128],
                                                 rhs, start=(sh == 0), stop=(sh == 24))
                            nc.scalar.activation(
                                mkap(co, y0 * 28, [[1, n]]),
                                pt[:, :n], AF.Relu,
                                bias=sc2[m][:, 1:2], scale=sc2[m][:, 0:1])
                        # pool2 -> c3in interior
                        p2 = act.tile([128, 27 * 13], F32, tag="p2", bufs=2)
                        max3(mkap(p2, 0, [[13, 27], [1, 13]]),
                             lambda o: mkap(co, o, [[28, 27], [2, 13]]), 1)
                        c3wv = mkap(c3in[g][m], islot * 240 + 17, [[1, 13], [16, 13]])
                        c3rv = mkap(c3in[g][m].bitcast(F32), islot * 240 + 17,
                                    [[1, 13], [16, 13]])
                        nc.vector.tensor_max(c3wv, mkap(p2, 0, [[1, 13], [26, 13]]),
                                             mkap(p2, 13, [[1, 13], [26, 13]]))
                        nc.vector.tensor_max(c3wv, c3rv,
                                             mkap(p2, 26, [[1, 13], [26, 13]]))

                # ---- layer-wise conv3/4/5 over the 4 image-pairs ----
                def make_padded(tag, nblk):
                    outs = {g: [act.tile([128, 2 * 240], F32R,
                                         tag=f"{tag}{m}_{g}", name=f"{tag}{m}_{g}")
                                for m in range(nblk)] for g in range(4)}
                    for g in range(4):
                        for t in outs[g]:
                            zfill(t, 2 * 240)
                    return outs

                def convq(cins, w_d, sc, nkb, M, write_fn):
                    for m in range(M // 128):
                        wts = []
                        for sh in range(9):
                            row = []
                            for kb in range(nkb):
                                wt = wq.tile([128, 128], F32R, tag="wq")
                                nc.sync.dma_start(
                                    out=wt,
                                    in_=w_d[sh, kb, :,
                                            m * 128:(m + 1) * 128].bitcast(F32R))
                                row.append(wt)
                            wts.append(row)
                        for g in range(4):
                            pt = ps.tile([128, 512], F32, tag="ps")
                            first = True
                            for sh in range(9):
                                r, s = divmod(sh, 3)
                                for kb in range(nkb):
                                    rhs = mkap(cins[g][kb], r * 16 + s,
                                               [[240, 2], [16, 13], [1, 14]])
                                    nc.tensor.matmul(
                                        pt[:, :364], wts[sh][kb], rhs,
                                        start=first,
                                        stop=(sh == 8 and kb == nkb - 1))
                                    first = False
                            write_fn(g, m, pt, sc)

                def write_pad(nxt):
                    def fn(g, m, pt, sc):
                        nc.scalar.activation(
                            mkap(nxt[g][m], 17, [[240, 2], [16, 13], [1, 13]]),
                            mkap(pt, 0, [[182, 2], [14, 13], [1, 13]]),
                            AF.Relu, bias=sc[m][:, 1:2], scale=sc[m][:, 0:1])
                    return fn

                c4in = make_padded("c4in", 3)
                convq(c3in, w3c_d, sc3, 2, 384, write_pad(c4in))
                c5in = make_padded("c5in", 3)
                convq(c4in, w4c_d, sc4, 3, 384, write_pad(c5in))

                def write_c5(g, m, pt, sc):
                    c5o = act.tile([128, 2 * 169], F32, tag="c5out", bufs=4)
                    nc.scalar.activation(
                        mkap(c5o, 0, [[169, 2], [13, 13], [1, 13]]),
                        mkap(pt, 0, [[182, 2], [14, 13], [1, 13]]),
                        AF.Relu, bias=sc[m][:, 1:2], scale=sc[m][:, 0:1])
                    p3a = act.tile([128, 2 * 13 * 6], F32, tag="p3a", bufs=2)
                    max3(mkap(p3a, 0, [[78, 2], [6, 13], [1, 6]]),
                         lambda o: mkap(c5o, o, [[169, 2], [13, 13], [2, 6]]), 1)
                    hst = act.tile([128, 2 * 36], F32, tag="hst", bufs=2)
                    max3(mkap(hst, 0, [[36, 2], [1, 6], [6, 6]]),
                         lambda o: mkap(p3a, o, [[78, 2], [1, 6], [12, 6]]), 6)
                    nc.sync.dma_start(
                        out=rawap(h_loc, (2 * g) * 9216 + m * 4608,
                                  [[36, 128], [9216, 2], [1, 36]]),
                        in_=hst.rearrange("p (i f) -> p i f", i=2))

                convq(c5in, w5c_d, sc5, 3, 256, write_c5)

            # ================= FC stack =================
            nc.gpsimd.collective_compute(
                "AllGather", mybir.AluOpType.bypass,
                replica_groups=[list(range(NCORES))],
                ins=[h_loc[:, :]], outs=[h_all[:, :]])

            with tc.tile_pool(name="fcw", bufs=12) as fcw, \
                 tc.tile_pool(name="fca", bufs=1) as fca, \
                 tc.tile_pool(name="ps2", bufs=2, space="PSUM") as ps2, \
                 tc.tile_pool(name="psm", bufs=1, space="PSUM") as psm:

                h_sb = fca.tile([64, 9216], F32)
                nc.sync.dma_start(out=h_sb, in_=h_all)

                # fc1: y1T[o, img] = relu(E1 * (sgn(fw1) @ hT) + fb1)
                hts = []
                for kt in range(72):
                    pt = ps2.tile([128, 64], F32, tag="ptr")
                    nc.tensor.transpose(pt, h_sb[:, kt * 128:(kt + 1) * 128], idn)
                    ht = fca.tile([128, 64], F32R, name=f"hT{kt}")
                    nc.scalar.activation(ht, pt, AF.Copy)
                    hts.append(ht)

                pms = [psm.tile([128, 64], F32, tag=f"pm{m}", name=f"pm1_{m}")
                       for m in range(4)]
                for kt in range(72):
                    for m in range(4):
                        wt = fcw.tile([128, 128], F32R, tag="fw")
                        nc.sync.dma_start(
                            out=wt, in_=fw1_d[kt * 128:(kt + 1) * 128,
                                             m * 128:(m + 1) * 128].bitcast(F32R))
                        nc.tensor.matmul(pms[m], wt, hts[kt],
                                         start=(kt == 0), stop=(kt == 71))
                for m in range(4):
                    y1t = fca.tile([128, 64], F32R, name=f"y1t{m}")
                    nc.scalar.activation(y1t, pms[m], AF.Relu,
                                         bias=sf1[m][:, 1:2], scale=sf1[m][:, 0:1])
                    nc.sync.dma_start(out=y1loc[m * 128:(m + 1) * 128, :].bitcast(F32R),
                                      in_=y1t)

                nc.gpsimd.collective_compute(
                    "AllGather", mybir.AluOpType.bypass,
                    replica_groups=[list(range(NCORES))],
                    ins=[y1loc[:, :]], outs=[y1all[:, :]])

                pms2 = [psm.tile([128, 64], F32, tag=f"pm{m}", name=f"pm2_{m}")
                        for m in range(4)]
                for kt in range(32):
                    yt = fcw.tile([128, 64], F32R, tag="yt")
                    nc.sync.dma_start(
                        out=yt, in_=y1all[kt * 128:(kt + 1) * 128, :].bitcast(F32R))
                    for m in range(4):
                        wt = fcw.tile([128, 128], F32R, tag="fw")
                        nc.sync.dma_start(
                            out=wt, in_=fw2_d[kt * 128:(kt + 1) * 128,
                                              m * 128:(m + 1) * 128].bitcast(F32R))
                        nc.tensor.matmul(pms2[m], wt, yt,
                                         start=(kt == 0), stop=(kt == 31))
                for m in range(4):
                    y2t = fca.tile([128, 64], F32R, name=f"y2t{m}")
                    nc.scalar.activation(y2t, pms2[m], AF.Relu,
                                         bias=sf2[m][:, 1:2], scale=sf2[m][:, 0:1])
                    nc.sync.dma_start(out=y2loc[m * 128:(m + 1) * 128, :].bitcast(F32R),
                                      in_=y2t)

                nc.gpsimd.collective_compute(
                    "AllGather", mybir.AluOpType.bypass,
                    replica_groups=[list(range(NCORES))],
                    ins=[y2loc[:, :]], outs=[y2all[:, :]])

                pm3 = psm.tile([126, 64], F32, tag="pm3", name="pm3")
                for kt in range(32):
                    yt = fcw.tile([128, 64], F32, tag="yt3")
                    nc.sync.dma_start(out=yt, in_=y2all[kt * 128:(kt + 1) * 128, :])
                    wt = fcw.tile([128, 126], F32, tag="fw3")
                    nc.sync.dma_start(out=wt, in_=fw3_d[kt * 128:(kt + 1) * 128, :])
                    nc.tensor.matmul(pm3, wt, yt, start=(kt == 0), stop=(kt == 31))
                osb = fca.tile([126, 64], F32, name="osb")
                nc.vector.tensor_scalar_add(osb, pm3, sf3[0][:126, 1:2])
                nc.sync.dma_start(out=out_d, in_=osb)

    nc.compile()
    return nc


def _get_program():
    if "nc" not in _PROG_CACHE:
        _PROG_CACHE["nc"] = _build_program()
    return _PROG_CACHE["nc"]


def _host_prep(inputs):
    eps = 1e-5
    f32 = np.float32

    def inv(g, v):
        return (g / np.sqrt(v + eps)).astype(f32)

    def rms(w):
        return np.sqrt(np.mean(w.astype(np.float64) ** 2)).astype(f32)

    x = inputs["x"]
    w1, b1 = inputs["w1"], inputs["b1"]
    inv1 = inv(inputs["g1"], inputs["v1"])
    w1f = (w1 * inv1[:, None, None, None]).astype(f32)
    b1f = (b1 * inv1 + inputs["be1"] - inputs["m1"] * inv1).astype(f32)

    # conv1 im2col: [B, 3, 121, 3025]
    xp = np.pad(x, ((0, 0), (0, 0), (2, 2), (2, 2)))
    s = xp.strides
    win = np.lib.stride_tricks.as_strided(
        xp, shape=(B, 3, 11, 11, 55, 55),
        strides=(s[0], s[1], s[2], s[3], 4 * s[2], 4 * s[3]))
    xcol = win.reshape(B, 3, 121, 3025).astype(f32)
    w1c = np.ascontiguousarray(
        w1f.reshape(96, 3, 121).transpose(1, 2, 0)).astype(f32)  # [3,121,96]
    sc1 = np.stack([np.ones(96, f32), b1f], axis=1)

    inv2 = inv(inputs["g2"], inputs["v2"])
    E2 = rms(inputs["w2"])
    sgn2 = np.sign(inputs["w2"]).astype(f32)  # [256, 96, 5, 5]
    w2c = np.ascontiguousarray(
        sgn2.reshape(256, 96, 25).transpose(2, 1, 0)).astype(f32)  # [25,96,256]
    sc2 = np.stack([(E2 * inv2).astype(f32),
                    (inputs["b2"] * inv2 + inputs["be2"]
                     - inputs["m2"] * inv2).astype(f32)], axis=1).astype(f32)

    def conv_sgn(w, nkb, M):
        sgn = np.sign(w).astype(f32)  # [M, K, 3, 3]
        K = sgn.shape[1]
        out = np.zeros((9, nkb, 128, M), f32)
        for r in range(3):
            for s_ in range(3):
                blk = sgn[:, :, r, s_].T  # [K, M]
                for kb in range(nkb):
                    kk = min(128, K - kb * 128)
                    out[r * 3 + s_, kb, :kk] = blk[kb * 128:kb * 128 + kk]
        return out

    E3, E4, E5 = rms(inputs["w3"]), rms(inputs["w4"]), rms(inputs["w5"])
    w3c = conv_sgn(inputs["w3"], 2, 384)
    sc3 = np.stack([np.full(384, E3, f32), inputs["b3"].astype(f32)], axis=1)
    w4c = conv_sgn(inputs["w4"], 3, 384)
    sc4 = np.stack([np.full(384, E4, f32), inputs["b4"].astype(f32)], axis=1)
    w5c = conv_sgn(inputs["w5"], 3, 256)
    sc5 = np.stack([np.full(256, E5, f32), inputs["b5"].astype(f32)], axis=1)

    Ef1, Ef2 = rms(inputs["fw1"]), rms(inputs["fw2"])
    sgnf1 = np.sign(inputs["fw1"]).astype(f32)
    sgnf2 = np.sign(inputs["fw2"]).astype(f32)

    shared = dict(w1c=w1c, sc1=sc1, w2c=w2c, sc2=sc2, w3c=w3c, sc3=sc3,
                  w4c=w4c, sc4=sc4, w5c=w5c, sc5=sc5)
    in_maps = []
    for c in range(NCORES):
        m = dict(shared)
        m["xcol"] = np.ascontiguousarray(xcol[c * BPC:(c + 1) * BPC])
        m["fw1t"] = np.ascontiguousarray(sgnf1[c * 512:(c + 1) * 512].T)
        m["sf1"] = np.stack([np.full(512, Ef1, f32),
                             inputs["fb1"][c * 512:(c + 1) * 512].astype(f32)],
                            axis=1)
        m["fw2t"] = np.ascontiguousarray(sgnf2[c * 512:(c + 1) * 512].T)
        m["sf2"] = np.stack([np.full(512, Ef2, f32),
                             inputs["fb2"][c * 512:(c + 1) * 512].astype(f32)],
                            axis=1)
        fw3s = np.zeros((4096, 126), f32)
        fw3s[:, :125] = inputs["fw3"][c * 125:(c + 1) * 125].T
        m["fw3t"] = fw3s
        fb3s = np.zeros(126, f32)
        fb3s[:125] = inputs["fb3"][c * 125:(c + 1) * 125]
        m["sf3"] = np.stack([np.ones(126, f32), fb3s], axis=1)
        in_maps.append(m)
    return in_maps


def kernel(**inputs):
    global LAST_EXEC_NS, LAST_RESULTS
    from concourse import bass_utils

    nc = _get_program()
    in_maps = _host_prep(inputs)
    trace = os.environ.get("BASS_KERNEL_TRACE", "0") == "1"
    res = bass_utils.run_bass_kernel_spmd(
        nc, in_maps, core_ids=list(range(NCORES)), trace=trace)
    LAST_EXEC_NS = res.exec_time_ns
    LAST_RESULTS = res

    out = np.zeros((B, 1000), np.float32)
    for c in range(NCORES):
        out[:, c * 125:(c + 1) * 125] = res.results[c]["out"][:125, :].T
    return out
